# revision 60
# baseline (speedup 1.0000x reference)
"""Trainium2 Bass kernel for nn_BRCLoss (supervised-contrastive style loss).

Math (per batch sample b, matching the jax reference):
    f = features[b].reshape(24, 4096); fhat = f / ||f||_row
    logits = (fhat @ fhat.T) / 0.1                       # [24, 24]
    exp_logits = exp(logits) * (1 - I)
    log_prob = logits - log(exp_logits.sum(-1))
    mlpp = (mask * log_prob).sum(-1) / (mask.sum(-1) + 1e-6)
    loss = sum_b mean_m(-0.1 * mlpp) / 512               # scalar

`outputs` / `targets` are unused by the reference; only `features`
[512, 2, 12, 4096] f32 matters.  Pure data parallel: 64 samples per core.

The problem is memory-bound: per core 24 MiB of f32 features must stream
from HBM (~67 us at the ~375 GB/s per-core roofline), against which the
useful output is just the per-sample [24,24] Gram blocks (0.3% of the
FLOPs live outside them).  The kernel therefore does exactly the
memory-bound part on device — stream, transpose, Gram, cast — and ships
the bf16 Gram tiles out; the O(B*M^2) scalar softmax/weighting tail runs
on the host in f64 (the earlier on-device epilogue ran straight into the
HAM power-management duty cycle: after ~45-50 us of full-rate streaming
the clock halves in 10-15 us windows, exactly when the end-of-stream
epilogue chain was exposed; dropping the epilogue also drops its energy).

Per-core kernel:
  - 12 tiles of [120 rows, 4096] (5 samples) + 1 tail tile of [96 rows]
    (4 samples) — 1536 rows exactly, nothing re-read.
  - ALL feature-load triggers are issued up front (fpool holds all 13
    tiles) so the 16 SWDGE engines never starve on descriptor supply.
    2048-column pieces give 8 KB descriptor rows (measured faster than
    16 KB rows).  Triggers past the 8-deep SWDGE completion-semaphore
    pool recycle-wait on earlier tiles' DMAs, which only stalls the
    otherwise idle gpsimd queue, never the stream.  The identity constant
    rides a scalar-HWDGE DMA issued before the feature triggers (a const
    DMA enqueued behind the saturated feature queue crawls for tens of
    microseconds).
  - Feature loads are SWDGE (gpsimd) DMAs that cast f32 -> bf16 in
    flight: HBM still reads the full f32 stream (the roofline), but SBUF
    writes halve, which relieves the port bottleneck shared with the
    sibling NeuronCore under 8-core SPMD.
  - Per tile: PE-transposes 32 bf16 chunks [R,128] -> PSUM (8 per bank,
    6 banks deep), copies them to SBUF (DVE + ACT split), then 32
    accumulating bf16 matmuls build the block-diagonal Gram G [R,R] (one
    [120,120] Gram covers 5 samples' [24,24] blocks; the off-diagonal
    blocks are never read).  One DVE cast to bf16 and one sync-HWDGE DMA
    ship each tile; the device tail after the last HBM byte is a single
    quad + cast + small DMA.
"""

import os
import sys

import numpy as np

if "/opt/trn_rl_repo" not in sys.path:
    sys.path.insert(0, "/opt/trn_rl_repo")

# Problem constants (hardcoded; kernel.py must be self-contained).
B = 512
NV = 2
NCLS = 12
D = 4096
M = NV * NCLS              # 24 anchor rows per sample
NCORES = 8
SPC = B // NCORES          # 64 samples per core
ROWS = SPC * M             # 1536 feature rows per core
P = 120                    # rows per full tile (5 samples)
T = 13                     # tiles per core: 12 full + 1 tail of 96 rows
PTAIL = ROWS - P * (T - 1)  # 96 rows (4 samples) in the tail tile
CH = 128                   # contraction chunk (PE partition limit)
NCH = D // CH              # 32 chunks
QUAD = 8                   # transposed chunks packed per PSUM bank
NQ = NCH // QUAD
TEMP = 0.1
EPS_POS = 1e-6

_compiled = None           # (nc, const_in_map)
LAST_RESULTS = None        # BassKernelResults of the most recent run


def _host_consts():
    import ml_dtypes

    ident = np.eye(128, dtype=np.float32).astype(ml_dtypes.bfloat16)
    return {"ident": ident}


def _build():
    from contextlib import ExitStack

    from concourse import bacc, bass, mybir, tile

    f32 = mybir.dt.float32
    bf16 = mybir.dt.bfloat16
    fp8 = mybir.dt.float8e4

    nc = bacc.Bacc("TRN2", target_bir_lowering=False, debug=False,
                   num_devices=NCORES)

    f_dram = nc.dram_tensor("f", (ROWS, D), f32, kind="ExternalInput")
    id_dram = nc.dram_tensor("ident", (128, 128), bf16, kind="ExternalInput")
    out_dram = nc.dram_tensor("gout", (ROWS, P), bf16, kind="ExternalOutput")

    ROWCNT = [P] * (T - 1) + [PTAIL]
    ROWOFF = [P * t for t in range(T)]
    PIECES = [[2048, 2048]] * (T - 1) + [[2048, 1024, 1024]]

    with ExitStack() as ctx:
        tc = ctx.enter_context(tile.TileContext(nc))
        consts = ctx.enter_context(tc.tile_pool(name="consts", bufs=1))
        fpool = ctx.enter_context(tc.tile_pool(name="fpool", bufs=T))
        tcpool = ctx.enter_context(tc.tile_pool(name="tcpool", bufs=5))
        egpool = ctx.enter_context(tc.tile_pool(name="egpool", bufs=4))
        tpsum = ctx.enter_context(
            tc.tile_pool(name="tpsum", bufs=6, space=bass.MemorySpace.PSUM))
        gpsum = ctx.enter_context(
            tc.tile_pool(name="gpsum", bufs=2, space=bass.MemorySpace.PSUM))

        # Identity first: it must clear the DMA engines before the feature
        # stream saturates them.
        identb = consts.tile([128, 128], bf16, tag="identb")
        nc.scalar.dma_start(identb[:], id_dram[:, :])

        ftiles = []
        for t in range(T):
            ftiles.append(fpool.tile([P, D], bf16, tag="f", name=f"ft{t}"))

        def load_tile(t):
            ft = ftiles[t]
            r0, rn = ROWOFF[t], ROWCNT[t]
            c0 = 0
            for w in PIECES[t]:
                nc.gpsimd.dma_start(ft[:rn, c0:c0 + w],
                                    f_dram[r0:r0 + rn, c0:c0 + w])
                c0 += w

        for t in range(T):
            load_tile(t)

        def tile_gram(t):
            """Transpose + Gram + ship for tile t."""
            ft = ftiles[t]
            rn = ROWCNT[t]
            g = gpsum.tile([P, P], f32, tag="g")
            tcs_list = []
            interleave = (t == T - 1)
            for q in range(NQ):
                # transposes land in PSUM as bf16 (unit stride); the copy
                # downcasts to fp8 so the Gram matmuls read/multiply half
                # the bytes — pure energy savings for the HAM power budget
                # at the (verified, ~4e-6 loss error) fp8 Gram precision
                tp = tpsum.tile([128, QUAD * P], bf16, tag="tp")
                tcs = tcpool.tile([128, QUAD * P], fp8, tag="tc")
                for j in range(QUAD):
                    c = q * QUAD + j
                    nc.tensor.transpose(
                        tp[:, j * P:j * P + rn],
                        ft[:rn, c * CH:(c + 1) * CH],
                        identb[:rn, :rn],
                    )
                # all copies on the DVE: it moves the same bytes in ~60% of
                # the ACT engine's time, and an idle scalar engine gives its
                # active-time energy back to the HAM power budget
                lo, hi = 0, (QUAD - 1) * P + rn
                nc.vector.tensor_copy(tcs[:, lo:hi], tp[:, lo:hi])
                if interleave:
                    for j in range(QUAD):
                        c = q * QUAD + j
                        sl = tcs[:, j * P:j * P + rn]
                        nc.tensor.matmul(g[:rn, :rn], sl, sl,
                                         start=(c == 0), stop=(c == NCH - 1))
                tcs_list.append(tcs)
            if not interleave:
                for c in range(NCH):
                    sl = tcs_list[c // QUAD][:, (c % QUAD) * P:(c % QUAD) * P + rn]
                    nc.tensor.matmul(g[:rn, :rn], sl, sl,
                                     start=(c == 0), stop=(c == NCH - 1))
            eg = egpool.tile([P, P], bf16, tag="eg")
            nc.vector.tensor_copy(eg[:rn, :rn], g[:rn, :rn])
            r0 = ROWOFF[t]
            nc.sync.dma_start(out_dram[r0:r0 + rn, 0:rn], eg[:rn, :rn])

        for t in range(T):
            tile_gram(t)

    nc.compile()
    return nc


def _host_loss(gblocks):
    """f64 softmax/weighting tail from the per-sample [24,24] Gram blocks.

    gblocks: [nsamples, 24, 24] float64 (bf16-rounded Grams).  Mirrors the
    reference exactly (is_stable=False log-softmax, +eps positive counts).
    """
    i = np.arange(NCLS)
    graph = (np.abs(i[:, None] - i[None, :]) <= 1).astype(np.float64)
    mask24 = np.tile(graph, (NV, NV)) * (1.0 - np.eye(M))
    d = np.sqrt(np.einsum("sii->si", gblocks))           # [S, 24] row norms
    logits = gblocks / (d[:, :, None] * d[:, None, :]) / TEMP
    el = np.exp(logits) * (1.0 - np.eye(M))
    log_prob = logits - np.log(el.sum(-1, keepdims=True))
    mlpp = (mask24 * log_prob).sum(-1) / (mask24.sum(-1) + EPS_POS)
    per_sample = (-TEMP * mlpp).mean(-1)                 # [S]
    return per_sample.sum() / B


def _ensure_axon_hooks():
    """Provide antenv.axon_hooks if the image lacks it (NTFF profiling shim).

    Mirrors trn_agent_boot.trn_boot: the hook drives NRT profiling via the
    libaxon_pjrt.so C ABI.  If anything is missing we register a None hook,
    which makes bass_utils skip tracing gracefully instead of crashing.
    """
    try:
        import antenv.axon_hooks  # noqa: F401
        return
    except ImportError:
        pass
    import contextlib
    import ctypes
    import types

    import antenv

    hook = None
    so_path = "/opt/axon/libaxon_pjrt.so"
    try:
        lib = ctypes.CDLL(so_path)
        if hasattr(lib, "axon_start_nrt_profile"):
            lib.axon_start_nrt_profile.argtypes = [
                ctypes.POINTER(ctypes.c_int64), ctypes.c_size_t]
            lib.axon_start_nrt_profile.restype = ctypes.c_int64
            lib.axon_stop_nrt_profile.argtypes = [ctypes.c_char_p]
            lib.axon_stop_nrt_profile.restype = ctypes.c_int64

            @contextlib.contextmanager
            def _hook(output_dir, device_ids):
                import jax
                jax.devices()
                if device_ids:
                    ids = (ctypes.c_int64 * len(device_ids))(*device_ids)
                    rc = lib.axon_start_nrt_profile(ids, len(device_ids))
                else:
                    rc = lib.axon_start_nrt_profile(None, 0)
                if rc != 0:
                    raise RuntimeError(f"axon_start_nrt_profile rc={rc}")
                try:
                    yield
                finally:
                    n = lib.axon_stop_nrt_profile(str(output_dir).encode())
                    print(f"profile: {n} file(s) written to {output_dir}",
                          file=sys.stderr)

            hook = _hook
    except OSError:
        pass

    mod = types.ModuleType("antenv.axon_hooks")
    state = {"hook": hook}
    mod.get_axon_ntff_profile_hook = lambda: state["hook"]
    mod.set_axon_ntff_profile_hook = lambda h: state.__setitem__("hook", h)
    sys.modules["antenv.axon_hooks"] = mod
    antenv.axon_hooks = mod


def kernel(**inputs):
    global _compiled, LAST_RESULTS
    from concourse import bass_utils

    feats = np.ascontiguousarray(
        np.asarray(inputs["features"], dtype=np.float32).reshape(B * M, D))

    if _compiled is None:
        _compiled = (_build(), _host_consts())
    nc, consts = _compiled

    in_maps = []
    for k in range(NCORES):
        im = dict(consts)
        im["f"] = feats[k * ROWS:(k + 1) * ROWS]
        in_maps.append(im)

    trace = bool(os.environ.get("BASS_TRACE"))
    if trace:
        _ensure_axon_hooks()
    try:
        res = bass_utils.run_bass_kernel_spmd(
            nc, in_maps, core_ids=list(range(NCORES)), trace=trace)
    except Exception:
        # Tracing plumbing or a transient device hiccup; retry once untraced.
        os.environ["BASS_NEVER_TRACE"] = "1"
        try:
            res = bass_utils.run_bass_kernel_spmd(
                nc, in_maps, core_ids=list(range(NCORES)), trace=False)
        finally:
            del os.environ["BASS_NEVER_TRACE"]
    LAST_RESULTS = res

    # Collect the diagonal [24,24] Gram blocks of every sample.
    ROWCNT = [P] * (T - 1) + [PTAIL]
    ROWOFF = [P * t for t in range(T)]
    blocks = []
    for r in res.results:
        gout = np.asarray(r["gout"], dtype=np.float64)   # [1536, 120]
        for t in range(T):
            r0, rn = ROWOFF[t], ROWCNT[t]
            gt = gout[r0:r0 + rn, 0:rn]
            for s in range(rn // M):
                blocks.append(gt[s * M:(s + 1) * M, s * M:(s + 1) * M])
    gblocks = np.stack(blocks)                           # [512, 24, 24]
    total = _host_loss(gblocks)
    return np.array(total, dtype=np.float32)


# revision 61
# speedup vs baseline: 1.1093x; 1.1093x over previous
"""Trainium2 Bass kernel for nn_BRCLoss (supervised-contrastive style loss).

Math (per batch sample b, matching the jax reference):
    f = features[b].reshape(24, 4096); fhat = f / ||f||_row
    logits = (fhat @ fhat.T) / 0.1                       # [24, 24]
    exp_logits = exp(logits) * (1 - I)
    log_prob = logits - log(exp_logits.sum(-1))
    mlpp = (mask * log_prob).sum(-1) / (mask.sum(-1) + 1e-6)
    loss = sum_b mean_m(-0.1 * mlpp) / 512               # scalar

`outputs` / `targets` are unused by the reference; only `features`
[512, 2, 12, 4096] f32 matters.  Pure data parallel: 64 samples per core.

The problem is memory-bound: per core 24 MiB of f32 features must stream
from HBM (~67 us at the ~375 GB/s per-core roofline), against which the
useful output is just the per-sample [24,24] Gram blocks (0.3% of the
FLOPs live outside them).  The kernel therefore does exactly the
memory-bound part on device — stream, transpose, Gram, cast — and ships
the bf16 Gram tiles out; the O(B*M^2) scalar softmax/weighting tail runs
on the host in f64 (the earlier on-device epilogue ran straight into the
HAM power-management duty cycle: after ~45-50 us of full-rate streaming
the clock halves in 10-15 us windows, exactly when the end-of-stream
epilogue chain was exposed; dropping the epilogue also drops its energy).

Per-core kernel:
  - 12 tiles of [120 rows, 4096] (5 samples) + 1 tail tile of [96 rows]
    (4 samples) — 1536 rows exactly, nothing re-read.
  - ALL feature-load triggers are issued up front (fpool holds all 13
    tiles) so the 16 SWDGE engines never starve on descriptor supply.
    2048-column pieces give 8 KB descriptor rows (measured faster than
    16 KB rows).  Triggers past the 8-deep SWDGE completion-semaphore
    pool recycle-wait on earlier tiles' DMAs, which only stalls the
    otherwise idle gpsimd queue, never the stream.  The identity constant
    rides a scalar-HWDGE DMA issued before the feature triggers (a const
    DMA enqueued behind the saturated feature queue crawls for tens of
    microseconds).
  - Feature loads are SWDGE (gpsimd) DMAs that cast f32 -> bf16 in
    flight: HBM still reads the full f32 stream (the roofline), but SBUF
    writes halve, which relieves the port bottleneck shared with the
    sibling NeuronCore under 8-core SPMD.
  - Per tile: PE-transposes 32 bf16 chunks [R,128] -> PSUM (8 per bank,
    6 banks deep), copies them to SBUF (DVE + ACT split), then 32
    accumulating bf16 matmuls build the block-diagonal Gram G [R,R] (one
    [120,120] Gram covers 5 samples' [24,24] blocks; the off-diagonal
    blocks are never read).  One DVE cast to bf16 and one sync-HWDGE DMA
    ship each tile; the device tail after the last HBM byte is a single
    quad + cast + small DMA.
"""

import os
import sys

import numpy as np

if "/opt/trn_rl_repo" not in sys.path:
    sys.path.insert(0, "/opt/trn_rl_repo")

# Problem constants (hardcoded; kernel.py must be self-contained).
B = 512
NV = 2
NCLS = 12
D = 4096
M = NV * NCLS              # 24 anchor rows per sample
NCORES = 8
SPC = B // NCORES          # 64 samples per core
ROWS = SPC * M             # 1536 feature rows per core
P = 120                    # rows per full tile (5 samples)
T = 13                     # tiles per core: 12 full + 1 tail of 96 rows
PTAIL = ROWS - P * (T - 1)  # 96 rows (4 samples) in the tail tile
CH = 128                   # contraction chunk (PE partition limit)
NCH = D // CH              # 32 chunks
QUAD = 8                   # transposed chunks packed per PSUM bank
NQ = NCH // QUAD
TEMP = 0.1
EPS_POS = 1e-6

_compiled = None           # (nc, const_in_map)
LAST_RESULTS = None        # BassKernelResults of the most recent run


def _host_consts():
    import ml_dtypes

    ident = np.eye(128, dtype=np.float32).astype(ml_dtypes.bfloat16)
    return {"ident": ident}


def _build():
    from contextlib import ExitStack

    from concourse import bacc, bass, mybir, tile

    f32 = mybir.dt.float32
    bf16 = mybir.dt.bfloat16
    fp8 = mybir.dt.float8e4

    nc = bacc.Bacc("TRN2", target_bir_lowering=False, debug=False,
                   num_devices=NCORES)

    f_dram = nc.dram_tensor("f", (ROWS, D), f32, kind="ExternalInput")
    id_dram = nc.dram_tensor("ident", (128, 128), bf16, kind="ExternalInput")
    out_dram = nc.dram_tensor("gout", (ROWS, P), bf16, kind="ExternalOutput")

    ROWCNT = [P] * (T - 1) + [PTAIL]
    ROWOFF = [P * t for t in range(T)]
    PIECES = [[2048, 2048]] * (T - 1) + [[2048, 1024, 1024]]

    with ExitStack() as ctx:
        tc = ctx.enter_context(tile.TileContext(nc))
        consts = ctx.enter_context(tc.tile_pool(name="consts", bufs=1))
        fpool = ctx.enter_context(tc.tile_pool(name="fpool", bufs=T))
        tcpool = ctx.enter_context(tc.tile_pool(name="tcpool", bufs=5))
        egpool = ctx.enter_context(tc.tile_pool(name="egpool", bufs=4))
        tpsum = ctx.enter_context(
            tc.tile_pool(name="tpsum", bufs=6, space=bass.MemorySpace.PSUM))
        gpsum = ctx.enter_context(
            tc.tile_pool(name="gpsum", bufs=2, space=bass.MemorySpace.PSUM))

        # Identity first: it must clear the DMA engines before the feature
        # stream saturates them.
        identb = consts.tile([128, 128], bf16, tag="identb")
        nc.scalar.dma_start(identb[:], id_dram[:, :])

        ftiles = []
        for t in range(T):
            ftiles.append(fpool.tile([P, D], bf16, tag="f", name=f"ft{t}"))

        def load_tile(t):
            ft = ftiles[t]
            r0, rn = ROWOFF[t], ROWCNT[t]
            c0 = 0
            for w in PIECES[t]:
                nc.gpsimd.dma_start(ft[:rn, c0:c0 + w],
                                    f_dram[r0:r0 + rn, c0:c0 + w])
                c0 += w

        for t in range(T):
            load_tile(t)

        def tile_gram(t):
            """Transpose + Gram + ship for tile t."""
            ft = ftiles[t]
            rn = ROWCNT[t]
            g = gpsum.tile([P, P], f32, tag="g")
            tcs_list = []
            interleave = (t == T - 1)
            for q in range(NQ):
                tp = tpsum.tile([128, QUAD * P], bf16, tag="tp")
                tcs = tcpool.tile([128, QUAD * P], bf16, tag="tc")
                for j in range(QUAD):
                    c = q * QUAD + j
                    nc.tensor.transpose(
                        tp[:, j * P:j * P + rn],
                        ft[:rn, c * CH:(c + 1) * CH],
                        identb[:rn, :rn],
                    )
                # all copies on the DVE: it moves the same bytes in ~60% of
                # the ACT engine's time, and an idle scalar engine gives its
                # active-time energy back to the HAM power budget
                lo, hi = 0, (QUAD - 1) * P + rn
                nc.vector.tensor_copy(tcs[:, lo:hi], tp[:, lo:hi])
                if interleave:
                    for j in range(QUAD):
                        c = q * QUAD + j
                        sl = tcs[:, j * P:j * P + rn]
                        nc.tensor.matmul(g[:rn, :rn], sl, sl,
                                         start=(c == 0), stop=(c == NCH - 1))
                tcs_list.append(tcs)
            if not interleave:
                for c in range(NCH):
                    sl = tcs_list[c // QUAD][:, (c % QUAD) * P:(c % QUAD) * P + rn]
                    nc.tensor.matmul(g[:rn, :rn], sl, sl,
                                     start=(c == 0), stop=(c == NCH - 1))
            eg = egpool.tile([P, P], bf16, tag="eg")
            nc.vector.tensor_copy(eg[:rn, :rn], g[:rn, :rn])
            r0 = ROWOFF[t]
            nc.sync.dma_start(out_dram[r0:r0 + rn, 0:rn], eg[:rn, :rn])

        for t in range(T):
            tile_gram(t)

    nc.compile()
    return nc


def _host_loss(gblocks):
    """f64 softmax/weighting tail from the per-sample [24,24] Gram blocks.

    gblocks: [nsamples, 24, 24] float64 (bf16-rounded Grams).  Mirrors the
    reference exactly (is_stable=False log-softmax, +eps positive counts).
    """
    i = np.arange(NCLS)
    graph = (np.abs(i[:, None] - i[None, :]) <= 1).astype(np.float64)
    mask24 = np.tile(graph, (NV, NV)) * (1.0 - np.eye(M))
    d = np.sqrt(np.einsum("sii->si", gblocks))           # [S, 24] row norms
    logits = gblocks / (d[:, :, None] * d[:, None, :]) / TEMP
    el = np.exp(logits) * (1.0 - np.eye(M))
    log_prob = logits - np.log(el.sum(-1, keepdims=True))
    mlpp = (mask24 * log_prob).sum(-1) / (mask24.sum(-1) + EPS_POS)
    per_sample = (-TEMP * mlpp).mean(-1)                 # [S]
    return per_sample.sum() / B


def _ensure_axon_hooks():
    """Provide antenv.axon_hooks if the image lacks it (NTFF profiling shim).

    Mirrors trn_agent_boot.trn_boot: the hook drives NRT profiling via the
    libaxon_pjrt.so C ABI.  If anything is missing we register a None hook,
    which makes bass_utils skip tracing gracefully instead of crashing.
    """
    try:
        import antenv.axon_hooks  # noqa: F401
        return
    except ImportError:
        pass
    import contextlib
    import ctypes
    import types

    import antenv

    hook = None
    so_path = "/opt/axon/libaxon_pjrt.so"
    try:
        lib = ctypes.CDLL(so_path)
        if hasattr(lib, "axon_start_nrt_profile"):
            lib.axon_start_nrt_profile.argtypes = [
                ctypes.POINTER(ctypes.c_int64), ctypes.c_size_t]
            lib.axon_start_nrt_profile.restype = ctypes.c_int64
            lib.axon_stop_nrt_profile.argtypes = [ctypes.c_char_p]
            lib.axon_stop_nrt_profile.restype = ctypes.c_int64

            @contextlib.contextmanager
            def _hook(output_dir, device_ids):
                import jax
                jax.devices()
                if device_ids:
                    ids = (ctypes.c_int64 * len(device_ids))(*device_ids)
                    rc = lib.axon_start_nrt_profile(ids, len(device_ids))
                else:
                    rc = lib.axon_start_nrt_profile(None, 0)
                if rc != 0:
                    raise RuntimeError(f"axon_start_nrt_profile rc={rc}")
                try:
                    yield
                finally:
                    n = lib.axon_stop_nrt_profile(str(output_dir).encode())
                    print(f"profile: {n} file(s) written to {output_dir}",
                          file=sys.stderr)

            hook = _hook
    except OSError:
        pass

    mod = types.ModuleType("antenv.axon_hooks")
    state = {"hook": hook}
    mod.get_axon_ntff_profile_hook = lambda: state["hook"]
    mod.set_axon_ntff_profile_hook = lambda h: state.__setitem__("hook", h)
    sys.modules["antenv.axon_hooks"] = mod
    antenv.axon_hooks = mod


def kernel(**inputs):
    global _compiled, LAST_RESULTS
    from concourse import bass_utils

    feats = np.ascontiguousarray(
        np.asarray(inputs["features"], dtype=np.float32).reshape(B * M, D))

    if _compiled is None:
        _compiled = (_build(), _host_consts())
    nc, consts = _compiled

    in_maps = []
    for k in range(NCORES):
        im = dict(consts)
        im["f"] = feats[k * ROWS:(k + 1) * ROWS]
        in_maps.append(im)

    trace = bool(os.environ.get("BASS_TRACE"))
    if trace:
        _ensure_axon_hooks()
    try:
        res = bass_utils.run_bass_kernel_spmd(
            nc, in_maps, core_ids=list(range(NCORES)), trace=trace)
    except Exception:
        # Tracing plumbing or a transient device hiccup; retry once untraced.
        os.environ["BASS_NEVER_TRACE"] = "1"
        try:
            res = bass_utils.run_bass_kernel_spmd(
                nc, in_maps, core_ids=list(range(NCORES)), trace=False)
        finally:
            del os.environ["BASS_NEVER_TRACE"]
    LAST_RESULTS = res

    # Collect the diagonal [24,24] Gram blocks of every sample.
    ROWCNT = [P] * (T - 1) + [PTAIL]
    ROWOFF = [P * t for t in range(T)]
    blocks = []
    for r in res.results:
        gout = np.asarray(r["gout"], dtype=np.float64)   # [1536, 120]
        for t in range(T):
            r0, rn = ROWOFF[t], ROWCNT[t]
            gt = gout[r0:r0 + rn, 0:rn]
            for s in range(rn // M):
                blocks.append(gt[s * M:(s + 1) * M, s * M:(s + 1) * M])
    gblocks = np.stack(blocks)                           # [512, 24, 24]
    total = _host_loss(gblocks)
    return np.array(total, dtype=np.float32)


# revision 63
# speedup vs baseline: 1.1233x; 1.0126x over previous
"""Trainium2 Bass kernel for nn_BRCLoss (supervised-contrastive style loss).

Math (per batch sample b, matching the jax reference):
    f = features[b].reshape(24, 4096); fhat = f / ||f||_row
    logits = (fhat @ fhat.T) / 0.1                       # [24, 24]
    exp_logits = exp(logits) * (1 - I)
    log_prob = logits - log(exp_logits.sum(-1))
    mlpp = (mask * log_prob).sum(-1) / (mask.sum(-1) + 1e-6)
    loss = sum_b mean_m(-0.1 * mlpp) / 512               # scalar

`outputs` / `targets` are unused by the reference; only `features`
[512, 2, 12, 4096] f32 matters.  Pure data parallel: 64 samples per core.

The problem is memory-bound: per core 24 MiB of f32 features must stream
from HBM (~67 us at the ~375 GB/s per-core roofline), against which the
useful output is just the per-sample [24,24] Gram blocks (0.3% of the
FLOPs live outside them).  The kernel therefore does exactly the
memory-bound part on device — stream, transpose, Gram, cast — and ships
the bf16 Gram tiles out; the O(B*M^2) scalar softmax/weighting tail runs
on the host in f64 (the earlier on-device epilogue ran straight into the
HAM power-management duty cycle: after ~45-50 us of full-rate streaming
the clock halves in 10-15 us windows, exactly when the end-of-stream
epilogue chain was exposed; dropping the epilogue also drops its energy).

Per-core kernel:
  - 12 tiles of [120 rows, 4096] (5 samples) + 1 tail tile of [96 rows]
    (4 samples) — 1536 rows exactly, nothing re-read.
  - ALL feature-load triggers are issued up front (fpool holds all 13
    tiles) so the 16 SWDGE engines never starve on descriptor supply.
    2048-column pieces give 8 KB descriptor rows (measured faster than
    16 KB rows).  Triggers past the 8-deep SWDGE completion-semaphore
    pool recycle-wait on earlier tiles' DMAs, which only stalls the
    otherwise idle gpsimd queue, never the stream.  The identity constant
    rides a scalar-HWDGE DMA issued before the feature triggers (a const
    DMA enqueued behind the saturated feature queue crawls for tens of
    microseconds).
  - Feature loads are SWDGE (gpsimd) DMAs that cast f32 -> bf16 in
    flight: HBM still reads the full f32 stream (the roofline), but SBUF
    writes halve, which relieves the port bottleneck shared with the
    sibling NeuronCore under 8-core SPMD.
  - Per tile: PE-transposes 32 bf16 chunks [R,128] -> PSUM (8 per bank,
    6 banks deep), copies them to SBUF, then 32 accumulating bf16 matmuls
    build the block-diagonal Gram G [R,R] (one [120,120] Gram covers 5
    samples' [24,24] blocks; the off-diagonal blocks are never read).
    ALL PSUM->SBUF copies ride the DVE: it moves the same bytes in ~60%
    of the ACT engine's time, and with the epilogue gone the scalar
    engine then runs ZERO instructions — its active-time energy goes
    back to the HAM power budget, which measurably delays the duty
    cycling (worth ~6 us end to end).  bf16 everywhere: fp8 was tried
    three ways (full pipeline, mixed-dtype transpose, fp8 matmul
    operands) and is either API-blocked or slower.
  - One DVE cast to bf16 and one sync-HWDGE DMA ship each tile; the
    device tail after the last HBM byte is a single quad + cast + small
    DMA (~3-5 us), and the rest is the fixed NEFF semaphore teardown.
"""

import os
import sys

import numpy as np

if "/opt/trn_rl_repo" not in sys.path:
    sys.path.insert(0, "/opt/trn_rl_repo")

# Problem constants (hardcoded; kernel.py must be self-contained).
B = 512
NV = 2
NCLS = 12
D = 4096
M = NV * NCLS              # 24 anchor rows per sample
NCORES = 8
SPC = B // NCORES          # 64 samples per core
ROWS = SPC * M             # 1536 feature rows per core
P = 120                    # rows per full tile (5 samples)
T = 13                     # tiles per core: 12 full + 1 tail of 96 rows
PTAIL = ROWS - P * (T - 1)  # 96 rows (4 samples) in the tail tile
CH = 128                   # contraction chunk (PE partition limit)
NCH = D // CH              # 32 chunks
QUAD = 8                   # transposed chunks packed per PSUM bank
NQ = NCH // QUAD
TEMP = 0.1
EPS_POS = 1e-6

_compiled = None           # (nc, const_in_map)
LAST_RESULTS = None        # BassKernelResults of the most recent run


def _host_consts():
    import ml_dtypes

    ident = np.eye(128, dtype=np.float32).astype(ml_dtypes.bfloat16)
    return {"ident": ident}


def _build():
    from contextlib import ExitStack

    from concourse import bacc, bass, mybir, tile

    f32 = mybir.dt.float32
    bf16 = mybir.dt.bfloat16

    nc = bacc.Bacc("TRN2", target_bir_lowering=False, debug=False,
                   num_devices=NCORES)

    f_dram = nc.dram_tensor("f", (ROWS, D), f32, kind="ExternalInput")
    id_dram = nc.dram_tensor("ident", (128, 128), bf16, kind="ExternalInput")
    out_dram = nc.dram_tensor("gout", (ROWS, P), bf16, kind="ExternalOutput")

    ROWCNT = [P] * (T - 1) + [PTAIL]
    ROWOFF = [P * t for t in range(T)]
    PIECES = [[2048, 2048]] * (T - 1) + [[2048, 1024, 1024]]

    with ExitStack() as ctx:
        tc = ctx.enter_context(tile.TileContext(nc))
        consts = ctx.enter_context(tc.tile_pool(name="consts", bufs=1))
        fpool = ctx.enter_context(tc.tile_pool(name="fpool", bufs=T))
        tcpool = ctx.enter_context(tc.tile_pool(name="tcpool", bufs=5))
        egpool = ctx.enter_context(tc.tile_pool(name="egpool", bufs=4))
        tpsum = ctx.enter_context(
            tc.tile_pool(name="tpsum", bufs=6, space=bass.MemorySpace.PSUM))
        gpsum = ctx.enter_context(
            tc.tile_pool(name="gpsum", bufs=2, space=bass.MemorySpace.PSUM))

        # Identity first: it must clear the DMA engines before the feature
        # stream saturates them.
        identb = consts.tile([128, 128], bf16, tag="identb")
        nc.scalar.dma_start(identb[:], id_dram[:, :])

        ftiles = []
        for t in range(T):
            ftiles.append(fpool.tile([P, D], bf16, tag="f", name=f"ft{t}"))

        def load_tile(t):
            ft = ftiles[t]
            r0, rn = ROWOFF[t], ROWCNT[t]
            c0 = 0
            for w in PIECES[t]:
                nc.gpsimd.dma_start(ft[:rn, c0:c0 + w],
                                    f_dram[r0:r0 + rn, c0:c0 + w])
                c0 += w

        for t in range(T):
            load_tile(t)

        def tile_gram(t):
            """Transpose + Gram + ship for tile t."""
            ft = ftiles[t]
            rn = ROWCNT[t]
            g = gpsum.tile([P, P], f32, tag="g")
            tcs_list = []
            interleave = (t == T - 1)
            for q in range(NQ):
                tp = tpsum.tile([128, QUAD * P], bf16, tag="tp")
                tcs = tcpool.tile([128, QUAD * P], bf16, tag="tc")
                for j in range(QUAD):
                    c = q * QUAD + j
                    nc.tensor.transpose(
                        tp[:, j * P:j * P + rn],
                        ft[:rn, c * CH:(c + 1) * CH],
                        identb[:rn, :rn],
                    )
                # all copies on the DVE: it moves the same bytes in ~60% of
                # the ACT engine's time, and an idle scalar engine gives its
                # active-time energy back to the HAM power budget
                lo, hi = 0, (QUAD - 1) * P + rn
                nc.vector.tensor_copy(tcs[:, lo:hi], tp[:, lo:hi])
                if interleave:
                    for j in range(QUAD):
                        c = q * QUAD + j
                        sl = tcs[:, j * P:j * P + rn]
                        nc.tensor.matmul(g[:rn, :rn], sl, sl,
                                         start=(c == 0), stop=(c == NCH - 1))
                tcs_list.append(tcs)
            if not interleave:
                for c in range(NCH):
                    sl = tcs_list[c // QUAD][:, (c % QUAD) * P:(c % QUAD) * P + rn]
                    nc.tensor.matmul(g[:rn, :rn], sl, sl,
                                     start=(c == 0), stop=(c == NCH - 1))
            eg = egpool.tile([P, P], bf16, tag="eg")
            nc.vector.tensor_copy(eg[:rn, :rn], g[:rn, :rn])
            r0 = ROWOFF[t]
            nc.sync.dma_start(out_dram[r0:r0 + rn, 0:rn], eg[:rn, :rn])

        for t in range(T):
            tile_gram(t)

    nc.compile()
    return nc


def _host_loss(gblocks):
    """f64 softmax/weighting tail from the per-sample [24,24] Gram blocks.

    gblocks: [nsamples, 24, 24] float64 (bf16-rounded Grams).  Mirrors the
    reference exactly (is_stable=False log-softmax, +eps positive counts).
    """
    i = np.arange(NCLS)
    graph = (np.abs(i[:, None] - i[None, :]) <= 1).astype(np.float64)
    mask24 = np.tile(graph, (NV, NV)) * (1.0 - np.eye(M))
    d = np.sqrt(np.einsum("sii->si", gblocks))           # [S, 24] row norms
    logits = gblocks / (d[:, :, None] * d[:, None, :]) / TEMP
    el = np.exp(logits) * (1.0 - np.eye(M))
    log_prob = logits - np.log(el.sum(-1, keepdims=True))
    mlpp = (mask24 * log_prob).sum(-1) / (mask24.sum(-1) + EPS_POS)
    per_sample = (-TEMP * mlpp).mean(-1)                 # [S]
    return per_sample.sum() / B


def _ensure_axon_hooks():
    """Provide antenv.axon_hooks if the image lacks it (NTFF profiling shim).

    Mirrors trn_agent_boot.trn_boot: the hook drives NRT profiling via the
    libaxon_pjrt.so C ABI.  If anything is missing we register a None hook,
    which makes bass_utils skip tracing gracefully instead of crashing.
    """
    try:
        import antenv.axon_hooks  # noqa: F401
        return
    except ImportError:
        pass
    import contextlib
    import ctypes
    import types

    import antenv

    hook = None
    so_path = "/opt/axon/libaxon_pjrt.so"
    try:
        lib = ctypes.CDLL(so_path)
        if hasattr(lib, "axon_start_nrt_profile"):
            lib.axon_start_nrt_profile.argtypes = [
                ctypes.POINTER(ctypes.c_int64), ctypes.c_size_t]
            lib.axon_start_nrt_profile.restype = ctypes.c_int64
            lib.axon_stop_nrt_profile.argtypes = [ctypes.c_char_p]
            lib.axon_stop_nrt_profile.restype = ctypes.c_int64

            @contextlib.contextmanager
            def _hook(output_dir, device_ids):
                import jax
                jax.devices()
                if device_ids:
                    ids = (ctypes.c_int64 * len(device_ids))(*device_ids)
                    rc = lib.axon_start_nrt_profile(ids, len(device_ids))
                else:
                    rc = lib.axon_start_nrt_profile(None, 0)
                if rc != 0:
                    raise RuntimeError(f"axon_start_nrt_profile rc={rc}")
                try:
                    yield
                finally:
                    n = lib.axon_stop_nrt_profile(str(output_dir).encode())
                    print(f"profile: {n} file(s) written to {output_dir}",
                          file=sys.stderr)

            hook = _hook
    except OSError:
        pass

    mod = types.ModuleType("antenv.axon_hooks")
    state = {"hook": hook}
    mod.get_axon_ntff_profile_hook = lambda: state["hook"]
    mod.set_axon_ntff_profile_hook = lambda h: state.__setitem__("hook", h)
    sys.modules["antenv.axon_hooks"] = mod
    antenv.axon_hooks = mod


def kernel(**inputs):
    global _compiled, LAST_RESULTS
    from concourse import bass_utils

    feats = np.ascontiguousarray(
        np.asarray(inputs["features"], dtype=np.float32).reshape(B * M, D))

    if _compiled is None:
        _compiled = (_build(), _host_consts())
    nc, consts = _compiled

    in_maps = []
    for k in range(NCORES):
        im = dict(consts)
        im["f"] = feats[k * ROWS:(k + 1) * ROWS]
        in_maps.append(im)

    trace = bool(os.environ.get("BASS_TRACE"))
    if trace:
        _ensure_axon_hooks()
    try:
        res = bass_utils.run_bass_kernel_spmd(
            nc, in_maps, core_ids=list(range(NCORES)), trace=trace)
    except Exception:
        # Tracing plumbing or a transient device hiccup; retry once untraced.
        os.environ["BASS_NEVER_TRACE"] = "1"
        try:
            res = bass_utils.run_bass_kernel_spmd(
                nc, in_maps, core_ids=list(range(NCORES)), trace=False)
        finally:
            del os.environ["BASS_NEVER_TRACE"]
    LAST_RESULTS = res

    # Collect the diagonal [24,24] Gram blocks of every sample.
    ROWCNT = [P] * (T - 1) + [PTAIL]
    ROWOFF = [P * t for t in range(T)]
    blocks = []
    for r in res.results:
        gout = np.asarray(r["gout"], dtype=np.float64)   # [1536, 120]
        for t in range(T):
            r0, rn = ROWOFF[t], ROWCNT[t]
            gt = gout[r0:r0 + rn, 0:rn]
            for s in range(rn // M):
                blocks.append(gt[s * M:(s + 1) * M, s * M:(s + 1) * M])
    gblocks = np.stack(blocks)                           # [512, 24, 24]
    total = _host_loss(gblocks)
    return np.array(total, dtype=np.float32)
